# revision 4
# baseline (speedup 1.0000x reference)
"""MoE (cosine-routed, top-k, 2-layer GELU FFN) on 8 Trainium2 NeuronCores.

Strategy (expert-parallel with F-split pairing):
  - Host computes the (tiny) routing: cosine scores -> softmax -> top-k ->
    renormalized gate weights. ~34 MFLOP, negligible vs the 34 GFLOP FFN.
  - Experts are sorted by token count and paired heavy/light. Core pair
    (2k, 2k+1) both handle experts (H[k], L[k]); core 2k computes the
    first half of D_FF, core 2k+1 the second half. Each core therefore
    runs tokens(H[k]) + tokens(L[k]) through an F/2-wide FFN: all cores
    execute an identical instruction stream with capacities (C1, C2) =
    (max heavy count, max light count).
  - The two F-halves of y = W2^T gelu(W1^T x + b1) + b2 are partial sums;
    b2 is added only in half 0. Each core scales its partial output by
    the token gate; the host scatter-adds everything (host work is not
    part of the measured HW exec time).

Pipeline design (v2), driven by the perfetto trace of v1:
  - NEFF init costs ~7us before any user instruction; teardown ~4us.
  - x and W1 are packed host-side into ONE "blob" DRAM tensor laid out in
    consumption order, so 4 large dma_starts (~0.64us issue cost each on
    the sync sequencer) replace 19 small ones and the first GEMM's data
    (x chunk0 + W1 f-block 0, front of blob) lands ~2.5us after issue.
  - W2 + gates go on the (otherwise idle until GEMM2) vector queue,
    gated behind a GEMM1 activation so their 4.4MB doesn't steal DMA
    bandwidth from the critical early x/W1 stream.
  - PE warm-up: 7 cold N=512 matmuls on a zeroed tile (~3us) bridge the
    gap until first data; HAM un-throttles (1.2->2.4GHz) ~3.4us after
    the first warm-up MM, just as real GEMMs begin.
  - Slot-0 chunking [336, 208]: equal-ish chunks cost 556 cyc per
    (f,d)-pair vs 578 for [512,32] (N=32 MMs pay a ~65-cycle dispatch
    floor), and the 336-wide chunk-0 sweep of f0 covers the transfer
    latency of the 208-wide x piece.
  - Output and gates are bf16 (tolerance is 2e-2; bf16 adds ~2e-3),
    halving out-DMA bytes. The last GEMM2 block is chunked [C-32, 32] so
    the final dependency chain after the last matmul is a 32-col vector
    op + a small DMA issued from the idle gpsimd queue.
"""

import numpy as np
import ml_dtypes

P = 128
D_MODEL = 1024
D_FF = 2048
N_EXPERTS = 8
N_CORES = 8
N_WARMUP_MM = 7

_BF16 = ml_dtypes.bfloat16

_cache: dict = {}
last_results = None  # BassKernelResults of the most recent run (for profiling)


def _chunks2(C):
    """Split C columns into <=512-wide near-equal chunks (16-aligned)."""
    if C <= 512:
        return [(0, C)]
    h = ((C // 2) + 15) // 16 * 16
    return [(0, h), (h, C - h)]


def _chunks_tail(C):
    """Chunking for the final GEMM2 block: leave a 32-wide tail."""
    if C <= 64:
        return [(0, C)]
    out = _chunks2(C - 32)
    return out + [(C - 32, 32)]


def _build(C1, C2):
    """Build + compile the SPMD paired-expert F-split FFN kernel."""
    import concourse.mybir as mybir
    from concourse import bacc
    from concourse.tile import TileContext

    D = D_MODEL
    ND = D // P             # 8 d-tiles
    NF1 = (D_FF // 2) // P  # 8 f-blocks per slot (F/2 = 1024)
    CS = [C1, C2]
    CK = [_chunks2(C1), _chunks2(C2)]
    W1B = ND * P            # columns per W1 f-block

    nc = bacc.Bacc("TRN2", target_bir_lowering=False, debug=False,
                   enable_partition_id=False)

    # ---- blob layout (host packs in this order; device slices it) ----
    # piece list: (name, width_cols)
    #   x slot0 chunk c: ND*cw cols (cols = d*cw + t within piece)
    #   w1 f-block fb:   W1B cols   (cols = d*P + j)
    #   x slot1 (single or per-chunk pieces, same d-within-chunk layout)
    blob_plan = []
    xoff = {}
    w1off = {}
    off = 0

    def add(name, w):
        nonlocal off
        blob_plan.append((name, off, w))
        if name.startswith("x"):
            xoff[name] = off
        else:
            w1off[name] = off
        off += w

    add("x00", ND * CK[0][0][1])
    add("w1_0", W1B)
    if len(CK[0]) > 1:
        add("x01", ND * CK[0][1][1])
    add("w1_1", W1B)
    add("w1_2", W1B)
    add("w1_3", W1B)
    for ci, (c0, cw) in enumerate(CK[1]):
        add(f"x1{ci}", ND * cw)
    for f in range(4, 2 * NF1):
        add(f"w1_{f}", W1B)
    BW = off  # total blob cols

    # issue boundaries: 4 dma_starts covering consumption-ordered ranges
    cut1 = xoff["x00"] + ND * CK[0][0][1] + W1B          # x00 + w1_0
    cut2 = w1off["w1_3"] + W1B                            # .. + x01,w1_1..3
    cut3 = w1off["w1_8"]                                  # .. + x1*, w1_4..7
    cuts = [0, cut1, cut2, cut3, BW]

    blob_d = nc.dram_tensor("blob", [P, BW], mybir.dt.bfloat16,
                            kind="ExternalInput")
    w2_d = nc.dram_tensor("w2", [P, 2 * NF1 * D], mybir.dt.bfloat16,
                          kind="ExternalInput")
    meta_d = nc.dram_tensor("meta", [P, 2 * NF1 + 2 * ND], mybir.dt.float32,
                            kind="ExternalInput")
    gate_d = nc.dram_tensor("gates", [P, C1 + C2], mybir.dt.bfloat16,
                            kind="ExternalInput")
    out_d = nc.dram_tensor("out", [D, C1 + C2], mybir.dt.bfloat16,
                           kind="ExternalOutput")

    OH = [0, NF1 * C1]       # ht col offset per slot
    OG = [0, C1]             # gate col offset per slot
    OO = [0, C1]             # out col offset per slot

    with TileContext(nc) as tc:
        with (
            tc.tile_pool(name="weights", bufs=1) as wp,
            tc.tile_pool(name="acts", bufs=1) as ap,
            tc.tile_pool(name="outs", bufs=4) as op,
            tc.tile_pool(name="psum", bufs=2, space="PSUM") as pp,
        ):
            # --- input DMAs first in program order on the sync queue.
            blob_t = wp.tile([P, BW], mybir.dt.bfloat16, tag="blob")
            for i in range(4):
                nc.sync.dma_start(out=blob_t[:, cuts[i] : cuts[i + 1]],
                                  in_=blob_d[:, cuts[i] : cuts[i + 1]])

            MW = 2 * NF1 + 2 * ND
            mt = wp.tile([P, MW], mybir.dt.float32, tag="meta")
            nc.scalar.dma_start(out=mt[:], in_=meta_d[:])
            b1t = mt[:, 0 : 2 * NF1]
            b2t = mt[:, 2 * NF1 : 2 * NF1 + 2 * ND]

            w2t = wp.tile([P, 2 * NF1 * D], mybir.dt.bfloat16, tag="w2")
            gt = wp.tile([P, C1 + C2], mybir.dt.bfloat16, tag="gates")
            ht = ap.tile([P, NF1 * (C1 + C2)], mybir.dt.bfloat16, tag="ht")

            # --- PE warm-up: cold N=512 matmuls on a zeroed tile bridge
            # the ~3us until the first x/W1 data lands, and trip the HAM
            # activity window so real GEMMs run at 2.4 GHz.
            dummy = ap.tile([P, 512], mybir.dt.bfloat16, tag="dummy")
            nc.gpsimd.memset(dummy[:], 0.0)
            wps = pp.tile([P, 512], mybir.dt.float32, tag="ps1_0",
                          name="warm_ps", bufs=2)
            for _ in range(N_WARMUP_MM):
                nc.tensor.matmul(wps[:], dummy[:, 0:P], dummy[:],
                                 start=True, stop=True)

            def x_ap(s, ci, c0, cw, d):
                o = xoff[f"x{s}{ci}"] + d * cw
                return blob_t[:, o : o + cw]

            def w1_ap(fb, d):
                o = w1off[f"w1_{fb}"] + d * P
                return blob_t[:, o : o + P]

            # --- GEMM1 + GELU. Slot-0 f0 runs chunk-outer (d-inner) so
            # the first matmuls need only x piece 0; everything after is
            # d-outer with chunks inner (weights stationary per (f,d)).
            gate_emitted = False
            for s in range(2):
                Cs, ck = CS[s], CK[s]
                for f in range(NF1):
                    fb = s * NF1 + f
                    ps = [pp.tile([P, cw], mybir.dt.float32, tag=f"ps1_{ci}",
                                  name=f"ps1_{fb}_{ci}", bufs=2)
                          for ci, (c0, cw) in enumerate(ck)]
                    if s == 0 and f == 0 and len(ck) > 1:
                        for ci, (c0, cw) in enumerate(ck):
                            for d in range(ND):
                                nc.tensor.matmul(
                                    ps[ci][:], w1_ap(fb, d), x_ap(s, ci, c0, cw, d),
                                    start=(d == 0), stop=(d == ND - 1))
                    else:
                        for d in range(ND):
                            lhs = w1_ap(fb, d)
                            for ci, (c0, cw) in enumerate(ck):
                                nc.tensor.matmul(
                                    ps[ci][:], lhs, x_ap(s, ci, c0, cw, d),
                                    start=(d == 0), stop=(d == ND - 1))
                    for ci, (c0, cw) in enumerate(ck):
                        nc.scalar.activation(
                            ht[:, OH[s] + f * Cs + c0 : OH[s] + f * Cs + c0 + cw],
                            ps[ci][:],
                            mybir.ActivationFunctionType.Gelu,
                            bias=b1t[:, fb : fb + 1],
                        )
                    if s == 0 and f == NF1 - 3 and not gate_emitted:
                        # release W2 + gates on the idle gpsimd queue once
                        # GEMM1 slot-0 is mostly fed (their 4.4MB would
                        # otherwise crowd the early x/W1 DMA stream).
                        gate_emitted = True
                        nc.gpsimd.tensor_copy(
                            dummy[:, 0:1],
                            ht[:, OH[0] + f * Cs : OH[0] + f * Cs + 1])
                        half = NF1 * D
                        nc.gpsimd.dma_start(out=w2t[:, :half], in_=w2_d[:, :half])
                        nc.gpsimd.dma_start(out=w2t[:, half:], in_=w2_d[:, half:])
                        nc.gpsimd.dma_start(out=gt[:], in_=gate_d[:])

            # --- GEMM2 + bias + gate per slot: yT[do*P:(do+1)*P, t].
            for s in range(2):
                Cs = CS[s]
                for do in range(ND):
                    last = s == 1 and do == ND - 1
                    ck2 = _chunks_tail(Cs) if last else CK[s]
                    ps2 = [pp.tile([P, cw], mybir.dt.float32, tag=f"ps2_{min(ci,1)}",
                                   name=f"ps2_{s}_{do}_{ci}",
                                   bufs=2)
                           for ci, (c0, cw) in enumerate(ck2)]
                    for f in range(NF1):
                        fb = s * NF1 + f
                        lhs = w2t[:, fb * D + do * P : fb * D + (do + 1) * P]
                        for ci, (c0, cw) in enumerate(ck2):
                            nc.tensor.matmul(
                                ps2[ci][:],
                                lhs,
                                ht[:, OH[s] + f * Cs + c0 : OH[s] + f * Cs + c0 + cw],
                                start=(f == 0),
                                stop=(f == NF1 - 1),
                            )
                    ot = op.tile([P, Cs], mybir.dt.bfloat16, tag="ot",
                                 name=f"ot_{s}_{do}")
                    for ci, (c0, cw) in enumerate(ck2):
                        nc.vector.scalar_tensor_tensor(
                            ot[:, c0 : c0 + cw],
                            ps2[ci][:],
                            b2t[:, s * ND + do : s * ND + do + 1],
                            gt[:, OG[s] + c0 : OG[s] + c0 + cw],
                            op0=mybir.AluOpType.add,
                            op1=mybir.AluOpType.mult,
                        )
                        eng = nc.gpsimd if (last and ci == len(ck2) - 1) else nc.sync
                        eng.dma_start(
                            out=out_d[do * P : (do + 1) * P,
                                      OO[s] + c0 : OO[s] + c0 + cw],
                            in_=ot[:, c0 : c0 + cw],
                        )

    nc.compile()
    return nc


def _get_kernel(C1, C2):
    if (C1, C2) not in _cache:
        _cache[(C1, C2)] = _build(C1, C2)
    return _cache[(C1, C2)]


def _run_spmd(nc, in_maps):
    """run_bass_kernel_spmd, robust to a BASS_TRACE env the image can't
    serve (missing antenv.axon_hooks / artifact upload): install a best-
    effort NTFF hook shim, and on a trace-path failure fall back to an
    untraced run."""
    import os
    from concourse.bass_utils import run_bass_kernel_spmd

    try:
        import antenv.axon_hooks  # noqa: F401
    except ImportError:
        import sys
        import types
        hook = None
        try:
            from trn_agent_boot.trn_boot import _ntff_profile_via_ctypes
            hook = _ntff_profile_via_ctypes("/opt/axon/libaxon_pjrt.so")
        except Exception:
            hook = None
        mod = types.ModuleType("antenv.axon_hooks")
        mod.get_axon_ntff_profile_hook = lambda: hook
        try:
            import antenv
            antenv.axon_hooks = mod
            sys.modules["antenv.axon_hooks"] = mod
        except ImportError:
            pass

    core_ids = list(range(N_CORES))
    try:
        return run_bass_kernel_spmd(nc, in_maps, core_ids)
    except Exception:
        if os.environ.get("BASS_NEVER_TRACE") == "1":
            raise
        os.environ["BASS_NEVER_TRACE"] = "1"
        try:
            return run_bass_kernel_spmd(nc, in_maps, core_ids)
        finally:
            del os.environ["BASS_NEVER_TRACE"]


def _pack_w1_half(W1e, h, NF1, ND):
    """-> [NF1 blocks][P, ND*P]: block f col layout d*P + j."""
    w = np.asarray(W1e[:, h * (D_FF // 2) : (h + 1) * (D_FF // 2)],
                   dtype=np.float32).astype(_BF16)
    # [D, F/2] -> [ND, P(d), NF1, P(f)] -> [NF1][P(d_in), ND, P(f_in)]
    # (contraction d_inner on partitions, as the matmul lhsT expects)
    return w.reshape(ND, P, NF1, P).transpose(2, 1, 0, 3)


def _pack_w2_half(W2e, h, NF1):
    w = np.asarray(W2e[h * (D_FF // 2) : (h + 1) * (D_FF // 2), :],
                   dtype=np.float32).astype(_BF16)
    return np.ascontiguousarray(
        w.reshape(NF1, P, D_MODEL).transpose(1, 0, 2).reshape(P, NF1 * D_MODEL))


def kernel(x, anchors, temperature, W1, b1, W2, b2, top_k):

    x = np.asarray(x)
    B, S, D = x.shape
    T = B * S
    E = np.asarray(anchors).shape[0]
    k = int(np.asarray(top_k))

    xf = np.ascontiguousarray(x.reshape(T, D), dtype=np.float32)

    # ---- routing on host (part of the dispatch decision) ----
    xn = xf / np.maximum(np.linalg.norm(xf, axis=-1, keepdims=True), 1e-8)
    an = np.asarray(anchors, dtype=np.float32)
    an = an / np.maximum(np.linalg.norm(an, axis=-1, keepdims=True), 1e-8)
    scores = (xn @ an.T) * abs(float(np.asarray(temperature)))
    scores -= scores.max(axis=-1, keepdims=True)
    probs = np.exp(scores)
    probs /= probs.sum(axis=-1, keepdims=True)
    topi = np.argsort(-probs, axis=-1, kind="stable")[:, :k]  # ties -> low idx
    topv = np.take_along_axis(probs, topi, axis=-1)
    gw = topv / (topv.sum(axis=-1, keepdims=True) + 1e-6)

    rows_per_e = []
    gates_per_e = []
    for e in range(E):
        mask = topi == e
        rows = np.nonzero(mask.any(axis=-1))[0]
        g = np.where(mask[rows], gw[rows], 0.0).sum(axis=-1).astype(np.float32)
        rows_per_e.append(rows)
        gates_per_e.append(g)

    # ---- pair heavy/light experts; 2 cores per pair split D_FF ----
    counts = np.array([len(r) for r in rows_per_e])
    order = np.argsort(-counts, kind="stable")
    heavy, light = order[: E // 2], order[E // 2 :]
    r32 = lambda n: max(64, -(-n // 32) * 32)
    C1 = r32(int(counts[heavy].max()))
    C2 = r32(int(counts[light].max()))
    nc = _get_kernel(C1, C2)

    ND, NF1 = D_MODEL // P, (D_FF // 2) // P
    x_bf = xf.astype(_BF16)
    CS = [C1, C2]
    CK = [_chunks2(C1), _chunks2(C2)]
    W1B = ND * P

    # blob column plan must mirror _build exactly
    blob_plan = []
    off = 0

    def add(name, w):
        nonlocal off
        blob_plan.append((name, off, w))
        off += w

    add("x00", ND * CK[0][0][1])
    add("w1_0", W1B)
    if len(CK[0]) > 1:
        add("x01", ND * CK[0][1][1])
    add("w1_1", W1B)
    add("w1_2", W1B)
    add("w1_3", W1B)
    for ci, (c0, cw) in enumerate(CK[1]):
        add(f"x1{ci}", ND * cw)
    for f in range(4, 2 * NF1):
        add(f"w1_{f}", W1B)
    BW = off

    def pack_x_piece(dst, o, c0, cw, rows):
        """x piece: [P, ND*cw], col = d*cw + t; tokens c0..c0+cw-1."""
        sel = rows[c0 : c0 + cw]
        n = len(sel)
        if n == 0:
            return
        xv = dst[:, o : o + ND * cw].reshape(P, ND, cw)
        xv[:, :, :n] = x_bf[sel].reshape(n, ND, P).transpose(2, 1, 0)

    in_maps = []
    for pair in range(E // 2):
        es = [int(heavy[pair]), int(light[pair])]
        for h in range(2):
            blob = np.zeros((P, BW), dtype=_BF16)
            w1blocks = [_pack_w1_half(np.asarray(W1[e]), h, NF1, ND)
                        for e in es]
            for name, o, w in blob_plan:
                if name.startswith("w1_"):
                    fb = int(name[3:])
                    blob[:, o : o + W1B] = (
                        w1blocks[fb // NF1][fb % NF1].reshape(P, W1B))
                else:
                    s, ci = int(name[1]), int(name[2])
                    c0, cw = CK[s][ci]
                    pack_x_piece(blob, o, c0, cw, rows_per_e[es[s]])
            w2 = np.concatenate(
                [_pack_w2_half(np.asarray(W2[e]), h, NF1) for e in es], axis=1)
            meta = np.zeros((P, 2 * NF1 + 2 * ND), dtype=np.float32)
            gates = np.zeros((P, C1 + C2), dtype=_BF16)
            for s, e in enumerate(es):
                b1h = np.asarray(b1[e], dtype=np.float32)[
                    h * (D_FF // 2) : (h + 1) * (D_FF // 2)]
                meta[:, s * NF1 : (s + 1) * NF1] = b1h.reshape(NF1, P).T
                if h == 0:  # b2 contributes once per expert
                    meta[:, 2 * NF1 + s * ND : 2 * NF1 + (s + 1) * ND] = (
                        np.asarray(b2[e], dtype=np.float32).reshape(ND, P).T)
                g0 = C1 if s else 0
                gates[:, g0 : g0 + len(rows_per_e[e])] = (
                    gates_per_e[e][None, :].astype(_BF16))
            in_maps.append({"blob": blob, "w2": w2, "meta": meta,
                            "gates": gates})

    res = _run_spmd(nc, in_maps)
    global last_results
    last_results = res

    # ---- combine (scatter-add the gated partial expert outputs) ----
    out = np.zeros((T, D_MODEL), dtype=np.float32)
    for pair in range(E // 2):
        es = [int(heavy[pair]), int(light[pair])]
        for h in range(2):
            o = res.results[2 * pair + h]["out"].astype(np.float32)
            for s, e in enumerate(es):
                rows = rows_per_e[e]
                n = len(rows)
                if n:
                    o0 = C1 if s else 0
                    out[rows] += o[:, o0 : o0 + n].T
    return out.reshape(B, S, D_MODEL)


# revision 5
# speedup vs baseline: 1.2846x; 1.2846x over previous
"""MoE (cosine-routed, top-k, 2-layer GELU FFN) on 8 Trainium2 NeuronCores.

Strategy (expert-parallel with F-split pairing):
  - Host computes the (tiny) routing: cosine scores -> softmax -> top-k ->
    renormalized gate weights. ~34 MFLOP, negligible vs the 34 GFLOP FFN.
  - Experts are sorted by token count and paired heavy/light. Core pair
    (2k, 2k+1) both handle experts (H[k], L[k]); core 2k computes the
    first half of D_FF, core 2k+1 the second half. Each core therefore
    runs tokens(H[k]) + tokens(L[k]) through an F/2-wide FFN: all cores
    execute an identical instruction stream with capacities (C1, C2) =
    (max heavy count, max light count).
  - The two F-halves of y = W2^T gelu(W1^T x + b1) + b2 are partial sums;
    b2 is added only in half 0. Each core scales its partial output by
    the token gate; the host scatter-adds everything (host work is not in
    the measured HW exec time).

Pipeline design (v3), driven by perfetto traces:
  - NEFF init costs ~7us before any user instruction; teardown ~4us
    (a trivial kernel measures 13.4us total).
  - DMA issue cost is ~0.65us per dma_start on a HWDGE sequencer, and a
    consumer of ANY sub-range of a transfer waits for the WHOLE transfer,
    so inputs stream as ~0.25-0.55MB dma_starts on the sync queue in
    consumption order: x-s0-piece0, w1 f0, x-s0-piece1, w1 f1..3, x-s1,
    w1 f4..15, then W2 and gates (needed only by GEMM2, ~30us later).
    gpsimd-queue DMAs are NOT used for inputs: their transfers fire as
    soon as descriptors arm (data deps only), stealing bandwidth.
  - PE warm-up: a few cold N=512 matmuls on a zeroed tile bridge the gap
    until the first data lands, and trip the HAM activity window so real
    GEMMs run at 2.4 GHz instead of 1.2.
  - Slot-0 (C1=544) chunking [272, 272]: equal chunks cost ~236ns per
    (f,d)-pair vs ~250 for [512, 32] (an N=32 matmul pays a ~65-cycle
    dispatch floor). f0 runs chunk-outer so its first matmuls need only
    x piece 0; later f-blocks run d-outer/chunk-inner so each stationary
    W1 tile serves both chunks.
  - Output and gates are bf16 (tolerance 2e-2; bf16 adds ~1e-3),
    halving out-DMA bytes. The final GEMM2 block is chunked
    [240, 240, 32] with the last 32-wide piece's DMA issued from the
    (idle at that point) scalar queue, so the post-last-matmul chain is
    two short vector ops + overlapped DMA issues.
"""

import numpy as np
import ml_dtypes

P = 128
D_MODEL = 1024
D_FF = 2048
N_EXPERTS = 8
N_CORES = 8
N_WARMUP_MM = 6

_BF16 = ml_dtypes.bfloat16

_cache: dict = {}
last_results = None  # BassKernelResults of the most recent run (for profiling)


def _chunks2(C):
    """Split C columns into <=512-wide near-equal chunks (16-aligned)."""
    if C <= 512:
        return [(0, C)]
    h = ((C // 2) + 15) // 16 * 16
    return [(0, h), (h, C - h)]


def _chunks_tail(C):
    """Chunking for the final GEMM2 block: fine-grained with a 32 tail."""
    if C <= 64:
        return [(0, C)]
    C0 = C - 32
    out = [(c0, cw) for c0, cw in _chunks2(C0)]
    if out[-1][1] > 272:
        c0, cw = out.pop()
        h = ((cw // 2) + 15) // 16 * 16
        out += [(c0, h), (c0 + h, cw - h)]
    return out + [(C0, 32)]


def _build(C1, C2):
    """Build + compile the SPMD paired-expert F-split FFN kernel."""
    import concourse.mybir as mybir
    from concourse import bacc
    from concourse.tile import TileContext

    D = D_MODEL
    ND = D // P             # 8 d-tiles
    NF1 = (D_FF // 2) // P  # 8 f-blocks per slot (F/2 = 1024)
    CS = [C1, C2]
    CK = [_chunks2(C1), _chunks2(C2)]
    W1B = ND * P            # columns per W1 f-block

    nc = bacc.Bacc("TRN2", target_bir_lowering=False, debug=False,
                   enable_partition_id=False)

    # x layout: per slot, chunk-major pieces: piece (s,ci) is [P, ND*cw]
    # with col = d*cw + t. Piece offsets within xT:
    xo = {}
    off = 0
    for s in range(2):
        for ci, (c0, cw) in enumerate(CK[s]):
            xo[(s, ci)] = off
            off += ND * cw
    XW = off

    xT_d = nc.dram_tensor("xT", [P, XW], mybir.dt.bfloat16,
                          kind="ExternalInput")
    w1_d = nc.dram_tensor("w1", [P, 2 * NF1 * W1B], mybir.dt.bfloat16,
                          kind="ExternalInput")
    w2_d = nc.dram_tensor("w2", [P, 2 * NF1 * D], mybir.dt.bfloat16,
                          kind="ExternalInput")
    meta_d = nc.dram_tensor("meta", [P, 2 * NF1 + 2 * ND], mybir.dt.float32,
                            kind="ExternalInput")
    gate_d = nc.dram_tensor("gates", [P, C1 + C2], mybir.dt.bfloat16,
                            kind="ExternalInput")
    out_d = nc.dram_tensor("out", [D, C1 + C2], mybir.dt.bfloat16,
                           kind="ExternalOutput")

    OH = [0, NF1 * C1]       # ht col offset per slot
    OG = [0, C1]             # gate col offset per slot
    OO = [0, C1]             # out col offset per slot

    with TileContext(nc) as tc:
        with (
            tc.tile_pool(name="weights", bufs=1) as wp,
            tc.tile_pool(name="acts", bufs=1) as ap,
            tc.tile_pool(name="outs", bufs=4) as op,
            tc.tile_pool(name="psum", bufs=2, space="PSUM") as pp,
        ):
            xt = ap.tile([P, XW], mybir.dt.bfloat16, tag="xt")
            w1t = wp.tile([P, 2 * NF1 * W1B], mybir.dt.bfloat16, tag="w1")
            w2t = wp.tile([P, 2 * NF1 * D], mybir.dt.bfloat16, tag="w2")
            MW = 2 * NF1 + 2 * ND
            mt = wp.tile([P, MW], mybir.dt.float32, tag="meta")
            b1t = mt[:, 0 : 2 * NF1]
            b2t = mt[:, 2 * NF1 : 2 * NF1 + 2 * ND]
            gt = wp.tile([P, C1 + C2], mybir.dt.bfloat16, tag="gates")
            ht = ap.tile([P, NF1 * (C1 + C2)], mybir.dt.bfloat16, tag="ht")

            def xdma(s, ci):
                o, w = xo[(s, ci)], ND * CK[s][ci][1]
                nc.sync.dma_start(out=xt[:, o : o + w], in_=xT_d[:, o : o + w])

            def wdma(fb):
                o = fb * W1B
                nc.sync.dma_start(out=w1t[:, o : o + W1B],
                                  in_=w1_d[:, o : o + W1B])

            # --- input DMAs in consumption-priority order (sync queue).
            xdma(0, 0)
            wdma(0)
            for ci in range(1, len(CK[0])):
                xdma(0, ci)
            wdma(1)
            wdma(2)
            wdma(3)
            for ci in range(len(CK[1])):
                xdma(1, ci)
            for f in range(4, 2 * NF1):
                wdma(f)
            NW2 = 4
            w2step = (2 * NF1 // NW2) * D
            for i in range(NW2):
                nc.sync.dma_start(out=w2t[:, i * w2step : (i + 1) * w2step],
                                  in_=w2_d[:, i * w2step : (i + 1) * w2step])
            nc.sync.dma_start(out=gt[:], in_=gate_d[:])
            nc.scalar.dma_start(out=mt[:], in_=meta_d[:])

            # --- PE warm-up: cold N=512 matmuls on a zeroed tile.
            dummy = ap.tile([P, 512], mybir.dt.bfloat16, tag="dummy")
            nc.gpsimd.memset(dummy[:], 0.0)
            wps = pp.tile([P, 512], mybir.dt.float32, tag="ps1_0",
                          name="warm_ps", bufs=2)
            for _ in range(N_WARMUP_MM):
                nc.tensor.matmul(wps[:], dummy[:, 0:P], dummy[:],
                                 start=True, stop=True)

            def x_ap(s, ci, cw, d):
                o = xo[(s, ci)] + d * cw
                return xt[:, o : o + cw]

            # --- GEMM1 + GELU. Slot-0 f0 runs chunk-outer (d-inner) so
            # its first matmuls need only x piece 0; everything else is
            # d-outer with chunks inner (W1 tile stationary per (f,d)).
            for s in range(2):
                Cs, ck = CS[s], CK[s]
                for f in range(NF1):
                    fb = s * NF1 + f
                    ps = [pp.tile([P, cw], mybir.dt.float32, tag=f"ps1_{ci}",
                                  name=f"ps1_{fb}_{ci}", bufs=2)
                          for ci, (c0, cw) in enumerate(ck)]
                    if s == 0 and f == 0 and len(ck) > 1:
                        for ci, (c0, cw) in enumerate(ck):
                            for d in range(ND):
                                nc.tensor.matmul(
                                    ps[ci][:],
                                    w1t[:, fb * W1B + d * P : fb * W1B + (d + 1) * P],
                                    x_ap(s, ci, cw, d),
                                    start=(d == 0), stop=(d == ND - 1))
                    else:
                        for d in range(ND):
                            lhs = w1t[:, fb * W1B + d * P : fb * W1B + (d + 1) * P]
                            for ci, (c0, cw) in enumerate(ck):
                                nc.tensor.matmul(
                                    ps[ci][:], lhs, x_ap(s, ci, cw, d),
                                    start=(d == 0), stop=(d == ND - 1))
                    for ci, (c0, cw) in enumerate(ck):
                        nc.scalar.activation(
                            ht[:, OH[s] + f * Cs + c0 : OH[s] + f * Cs + c0 + cw],
                            ps[ci][:],
                            mybir.ActivationFunctionType.Gelu,
                            bias=b1t[:, fb : fb + 1],
                        )

            # --- GEMM2 + bias + gate per slot: yT[do*P:(do+1)*P, t].
            for s in range(2):
                Cs = CS[s]
                for do in range(ND):
                    last = s == 1 and do == ND - 1
                    ck2 = _chunks_tail(Cs) if last else CK[s]
                    ps2 = [pp.tile([P, cw], mybir.dt.float32,
                                   tag=f"ps2_{ci % 2}",
                                   name=f"ps2_{s}_{do}_{ci}", bufs=2)
                           for ci, (c0, cw) in enumerate(ck2)]
                    for f in range(NF1):
                        fb = s * NF1 + f
                        lhs = w2t[:, fb * D + do * P : fb * D + (do + 1) * P]
                        for ci, (c0, cw) in enumerate(ck2):
                            nc.tensor.matmul(
                                ps2[ci][:],
                                lhs,
                                ht[:, OH[s] + f * Cs + c0 : OH[s] + f * Cs + c0 + cw],
                                start=(f == 0),
                                stop=(f == NF1 - 1),
                            )
                    ot = op.tile([P, Cs], mybir.dt.bfloat16, tag="ot",
                                 name=f"ot_{s}_{do}")
                    for ci, (c0, cw) in enumerate(ck2):
                        nc.vector.scalar_tensor_tensor(
                            ot[:, c0 : c0 + cw],
                            ps2[ci][:],
                            b2t[:, s * ND + do : s * ND + do + 1],
                            gt[:, OG[s] + c0 : OG[s] + c0 + cw],
                            op0=mybir.AluOpType.add,
                            op1=mybir.AluOpType.mult,
                        )
                        eng = nc.scalar if (last and ci == len(ck2) - 1) else nc.sync
                        eng.dma_start(
                            out=out_d[do * P : (do + 1) * P,
                                      OO[s] + c0 : OO[s] + c0 + cw],
                            in_=ot[:, c0 : c0 + cw],
                        )

    nc.compile()
    return nc


def _get_kernel(C1, C2):
    if (C1, C2) not in _cache:
        _cache[(C1, C2)] = _build(C1, C2)
    return _cache[(C1, C2)]


def _run_spmd(nc, in_maps):
    """run_bass_kernel_spmd, robust to a BASS_TRACE env the image can't
    serve (missing antenv.axon_hooks / artifact upload): install a best-
    effort NTFF hook shim, and on a trace-path failure fall back to an
    untraced run."""
    import os
    from concourse.bass_utils import run_bass_kernel_spmd

    try:
        import antenv.axon_hooks  # noqa: F401
    except ImportError:
        import sys
        import types
        hook = None
        try:
            from trn_agent_boot.trn_boot import _ntff_profile_via_ctypes
            hook = _ntff_profile_via_ctypes("/opt/axon/libaxon_pjrt.so")
        except Exception:
            hook = None
        mod = types.ModuleType("antenv.axon_hooks")
        mod.get_axon_ntff_profile_hook = lambda: hook
        try:
            import antenv
            antenv.axon_hooks = mod
            sys.modules["antenv.axon_hooks"] = mod
        except ImportError:
            pass

    core_ids = list(range(N_CORES))
    try:
        return run_bass_kernel_spmd(nc, in_maps, core_ids)
    except Exception:
        if os.environ.get("BASS_NEVER_TRACE") == "1":
            raise
        os.environ["BASS_NEVER_TRACE"] = "1"
        try:
            return run_bass_kernel_spmd(nc, in_maps, core_ids)
        finally:
            del os.environ["BASS_NEVER_TRACE"]


def _pack_w1_half(W1e, h, NF1, ND):
    """-> [P, 2*NF1*ND*P] layout: block fb at fb*ND*P, col d*P + f_in,
    partition = d_inner (contraction on partitions for matmul lhsT)."""
    w = np.asarray(W1e[:, h * (D_FF // 2) : (h + 1) * (D_FF // 2)],
                   dtype=np.float32).astype(_BF16)
    return np.ascontiguousarray(
        w.reshape(ND, P, NF1, P).transpose(1, 2, 0, 3).reshape(P, NF1 * ND * P))


def _pack_w2_half(W2e, h, NF1):
    w = np.asarray(W2e[h * (D_FF // 2) : (h + 1) * (D_FF // 2), :],
                   dtype=np.float32).astype(_BF16)
    return np.ascontiguousarray(
        w.reshape(NF1, P, D_MODEL).transpose(1, 0, 2).reshape(P, NF1 * D_MODEL))


def kernel(x, anchors, temperature, W1, b1, W2, b2, top_k):

    x = np.asarray(x)
    B, S, D = x.shape
    T = B * S
    E = np.asarray(anchors).shape[0]
    k = int(np.asarray(top_k))

    xf = np.ascontiguousarray(x.reshape(T, D), dtype=np.float32)

    # ---- routing on host (part of the dispatch decision) ----
    xn = xf / np.maximum(np.linalg.norm(xf, axis=-1, keepdims=True), 1e-8)
    an = np.asarray(anchors, dtype=np.float32)
    an = an / np.maximum(np.linalg.norm(an, axis=-1, keepdims=True), 1e-8)
    scores = (xn @ an.T) * abs(float(np.asarray(temperature)))
    scores -= scores.max(axis=-1, keepdims=True)
    probs = np.exp(scores)
    probs /= probs.sum(axis=-1, keepdims=True)
    topi = np.argsort(-probs, axis=-1, kind="stable")[:, :k]  # ties -> low idx
    topv = np.take_along_axis(probs, topi, axis=-1)
    gw = topv / (topv.sum(axis=-1, keepdims=True) + 1e-6)

    rows_per_e = []
    gates_per_e = []
    for e in range(E):
        mask = topi == e
        rows = np.nonzero(mask.any(axis=-1))[0]
        g = np.where(mask[rows], gw[rows], 0.0).sum(axis=-1).astype(np.float32)
        rows_per_e.append(rows)
        gates_per_e.append(g)

    # ---- pair heavy/light experts; 2 cores per pair split D_FF ----
    counts = np.array([len(r) for r in rows_per_e])
    order = np.argsort(-counts, kind="stable")
    heavy, light = order[: E // 2], order[E // 2 :]
    r32 = lambda n: max(64, -(-n // 32) * 32)
    C1 = r32(int(counts[heavy].max()))
    C2 = r32(int(counts[light].max()))
    nc = _get_kernel(C1, C2)

    ND, NF1 = D_MODEL // P, (D_FF // 2) // P
    x_bf = xf.astype(_BF16)
    CK = [_chunks2(C1), _chunks2(C2)]

    # x piece offsets must mirror _build
    xo = {}
    off = 0
    for s in range(2):
        for ci, (c0, cw) in enumerate(CK[s]):
            xo[(s, ci)] = off
            off += ND * cw
    XW = off

    def pack_x(dst, rows_s):
        for s in range(2):
            rows = rows_s[s]
            for ci, (c0, cw) in enumerate(CK[s]):
                sel = rows[c0 : c0 + cw]
                n = len(sel)
                if n == 0:
                    continue
                o = xo[(s, ci)]
                xv = dst[:, o : o + ND * cw].reshape(P, ND, cw)
                xv[:, :, :n] = x_bf[sel].reshape(n, ND, P).transpose(2, 1, 0)

    in_maps = []
    for pair in range(E // 2):
        es = [int(heavy[pair]), int(light[pair])]
        xT = np.zeros((P, XW), dtype=_BF16)
        pack_x(xT, [rows_per_e[es[0]], rows_per_e[es[1]]])
        for h in range(2):
            w1 = np.concatenate(
                [_pack_w1_half(np.asarray(W1[e]), h, NF1, ND) for e in es],
                axis=1)
            w2 = np.concatenate(
                [_pack_w2_half(np.asarray(W2[e]), h, NF1) for e in es], axis=1)
            meta = np.zeros((P, 2 * NF1 + 2 * ND), dtype=np.float32)
            gates = np.zeros((P, C1 + C2), dtype=_BF16)
            for s, e in enumerate(es):
                b1h = np.asarray(b1[e], dtype=np.float32)[
                    h * (D_FF // 2) : (h + 1) * (D_FF // 2)]
                meta[:, s * NF1 : (s + 1) * NF1] = b1h.reshape(NF1, P).T
                if h == 0:  # b2 contributes once per expert
                    meta[:, 2 * NF1 + s * ND : 2 * NF1 + (s + 1) * ND] = (
                        np.asarray(b2[e], dtype=np.float32).reshape(ND, P).T)
                g0 = C1 if s else 0
                gates[:, g0 : g0 + len(rows_per_e[e])] = (
                    gates_per_e[e][None, :].astype(_BF16))
            in_maps.append({"xT": xT, "w1": w1, "w2": w2, "meta": meta,
                            "gates": gates})

    res = _run_spmd(nc, in_maps)
    global last_results
    last_results = res

    # ---- combine (scatter-add the gated partial expert outputs) ----
    out = np.zeros((T, D_MODEL), dtype=np.float32)
    for pair in range(E // 2):
        es = [int(heavy[pair]), int(light[pair])]
        for h in range(2):
            o = res.results[2 * pair + h]["out"].astype(np.float32)
            for s, e in enumerate(es):
                rows = rows_per_e[e]
                n = len(rows)
                if n:
                    o0 = C1 if s else 0
                    out[rows] += o[:, o0 : o0 + n].T
    return out.reshape(B, S, D_MODEL)
